# revision 36
# baseline (speedup 1.0000x reference)
"""KMLayer (Kuramoto oscillator layer) on 8 Trainium2 NeuronCores via Bass/Tile.

Strategy (row-sharded, output-node parallel):
  - A = sc[0] * conn_w  [N,N] is row-sharded: core r owns rows m in
    [r*M_LOC, (r+1)*M_LOC).  The shard is premultiplied on the host, scaled
    by A_SCALE and cast to fp8-e4m3 (entries are ~1e-4; the descale rides
    the fused yt add), laid out partition-contiguous as A^T
    [n_lo=128 part, n_hi, m] so the kernel loads it with max-efficiency
    DMAs and keeps it RESIDENT in SBUF (8 MB/core).
  - The host also precomputes the cheap O(B*C*N) prep: groupnorm(c) -> y,
    normalize/transpose of x0, in the exact SBUF layouts the kernel uses.
  - Each Euler step: coup.T = X^T-stationary matmul over the SBUF-resident
    A^T shard (4-way PE column tiling), PE transposes fold [bc, m] ->
    [m, bc], then the per-row update (tangent projection, omega rotation,
    pair renormalize) runs on DVE/ACT for the local rows only, fused with
    zero-stride / pair-swap AP views.
  - The new local state slab (cast to fp8) is AllGather'd across the 8
    cores each step so every core has the full X for the next matmul.
  - Keep-warm matmuls hooked on vector-chain intermediates bridge the
    collective gap so the PE HAM clock-gate stays at full rate.
State is carried in fp32; only the matmul operands (A, gathered X) are fp8.
"""

import numpy as np
import ml_dtypes

import concourse.bass as bass
import concourse.mybir as mybir
import concourse.tile as tile
from concourse import bacc
from concourse.bass_utils import run_bass_kernel_spmd
from concourse.bass_interp import get_hw_module

F32 = mybir.dt.float32
BF16 = mybir.dt.bfloat16
F8 = mybir.dt.float8e4
ALU = mybir.AluOpType
ACTF = mybir.ActivationFunctionType
AXX = mybir.AxisListType.X

# fp8 matmul operands: halves the 16MB/exec A^T reload and the per-step
# gather payload.  A is scaled by A_SCALE on the host so its ~1e-4-magnitude
# entries sit in e4m3's normal range; the descale rides the fused yt add.
USE_FP8 = True
A_SCALE = 1024.0
MM_DT = F8 if USE_FP8 else BF16

N_CORES = 8
B, C, N_FULL = 2, 16, 8192
BC = B * C  # 32
Q_STEPS = 8
GN_EPS = 1e-5
NRM_EPS = 1e-6


def _raw(x):
    """Underlying 2D AP of a tile or AP."""
    return x[:, :]


def _pairb(x):
    """[p, g] -> [p, g, 2] zero-stride broadcast over the pair dim."""
    ap = _raw(x)
    return bass.AP(tensor=ap.tensor, offset=ap.offset, ap=list(ap.ap) + [[0, 2]])


def _pairswap(x):
    """[p, 2g] -> [p, g, 2] view with each pair's elements swapped."""
    ap = _raw(x)
    g = ap.ap[-1][1] // 2
    return bass.AP(tensor=ap.tensor, offset=ap.offset + 1,
                   ap=[list(ap.ap[0]), [2, g], [-1, 2]])


def build_program(n=N_FULL, ncores=N_CORES, q_steps=Q_STEPS, internal_steps=None,
                  ablate=()):
    # internal_steps: timing probe only — run that many Euler iterations
    # (output written for the first q_steps). Default = q_steps.
    # ablate: timing-sim probe only — {"coll", "warm", "stage"} to skip parts.
    if internal_steps is None:
        internal_steps = q_steps
    m_loc = n // ncores            # rows owned per core (1024)
    mch = m_loc // 128             # 128-row chunks per core (8)
    nch = n // 128                 # 128-col contraction chunks (64)
    fw = mch * BC                  # per-step local free width (256)
    mq = m_loc // 4                # m-range per PE column-tile group (256)
    rg = [list(range(ncores))]

    nc = bacc.Bacc("TRN2", target_bir_lowering=False, debug=False,
                   enable_asserts=False, num_devices=ncores)

    # ---- I/O (all pre-marshalled on host into SBUF-ready layouts) ----
    # small_i packs xl | y | omg(replicated) | gam | id32 | yT into one
    # tensor: fewer per-execute buffer bindings.  yT ([bc, m] layout, 32
    # partitions) feeds the eviction-fused "coup*descale + y" add.
    sw = 3 * fw + 1 + 32 + m_loc
    at_i = nc.dram_tensor("at_i", [128, nch * m_loc], MM_DT, kind="ExternalInput").ap()
    xg_i = nc.dram_tensor("xg_i", [128, nch * BC], MM_DT, kind="ExternalInput").ap()
    small_i = nc.dram_tensor("small_i", [128, sw], F32, kind="ExternalInput").ap()
    out_o = nc.dram_tensor("out_o", [128, q_steps * fw], F32,
                           kind="ExternalOutput").ap()

    with tile.TileContext(nc) as tc:
        with tc.tile_pool(name="consts", bufs=1) as consts, \
             tc.tile_pool(name="atbp", bufs=1) as atbp, \
             tc.tile_pool(name="state", bufs=2) as state, \
             tc.tile_pool(name="psmm", bufs=1, space="PSUM") as psmm, \
             tc.tile_pool(name="psf", bufs=2, space="PSUM") as psf, \
             tc.tile_pool(name="ew", bufs=2) as ew, \
             tc.tile_pool(name="agd", bufs=2, space="DRAM") as agd:

            # ---------------- constants & state loads ----------------
            small = consts.tile([128, sw], F32)
            nc.sync.dma_start(out=small, in_=small_i)
            xl_v = small[:, 0:fw]
            y_loc = small[:, fw:2 * fw]
            omg_sb = small[:, 2 * fw:3 * fw]
            gam_sb = small[:, 3 * fw:3 * fw + 1]
            id32_sb = small[0:32, 3 * fw + 1:3 * fw + 33]
            ylt_v = small[0:32, 3 * fw + 33:3 * fw + 33 + m_loc]
            eps6_sb = consts.tile([128, 1], F32)
            nc.vector.memset(eps6_sb, 2.0 * NRM_EPS)

            # persistent A^T shard [n_lo=128 part, (n_hi)(m_loc) free] bf16
            # (split into 8 DMAs so first-step matmuls overlap the load)
            atb = atbp.tile([128, nch * m_loc], MM_DT)
            ldc = nch * m_loc // 8
            for j in range(8):
                nc.sync.dma_start(out=atb[:, j * ldc:(j + 1) * ldc],
                                  in_=at_i[:, j * ldc:(j + 1) * ldc])
            atb_r = atb.rearrange("p (t m) -> p t m", m=m_loc)

            xcur = state.tile([128, nch * BC], MM_DT, tag="xcur")
            nc.sync.dma_start(out=xcur, in_=xg_i)
            xloc = state.tile([128, fw], F32, tag="xloc")
            nc.vector.tensor_copy(xloc, xl_v)
            # all steps' xn accumulate here; one contiguous DMA at the end
            outacc = consts.tile([128, q_steps * fw], F32)

            def pair_normalize(src, npairs, dst, pool):
                """dst = src / (||pair||+eps), src/dst [128, 2*npairs].
                sqrt(ss + 2e-6) ~= sqrt(ss) + 1e-6 near ss=1 (error << tol)."""
                sq = pool.tile([128, 2 * npairs], F32, tag="pn_sq")
                nc.vector.tensor_mul(sq, src, src)
                ss = pool.tile([128, npairs], F32, tag="pn_ss")
                nc.vector.tensor_reduce(
                    ss, sq.rearrange("p (g two) -> p g two", two=2),
                    axis=AXX, op=ALU.add)
                nr = pool.tile([128, npairs], F32, tag="pn_nr")
                nc.scalar.activation(out=nr, in_=ss, func=ACTF.Sqrt,
                                     bias=eps6_sb)
                rr = pool.tile([128, npairs], F32, tag="pn_rr")
                nc.vector.reciprocal_approx_fast(out=rr, in_=nr)
                sv = src.rearrange("p (g two) -> p g two", two=2)
                dv = dst.rearrange("p (g two) -> p g two", two=2)
                nc.vector.tensor_mul(dv, sv, _pairb(rr))
                return rr

            # ---------------- Euler steps ----------------
            if internal_steps == 0:
                for k in range(q_steps):
                    nc.vector.tensor_copy(outacc[:, k * fw:(k + 1) * fw], xloc)

            # scratch PSUM bank for keep-warm matmuls (never read back):
            # the PE HAM clock-gate re-throttles to 1.2 GHz after ~3.4us of
            # idle; the inter-step DVE/collective gap is ~15us, so without
            # fillers every step's matmuls run cold.  Junk matmuls hooked on
            # vector-chain intermediates + the gather staging keep it warm.
            psj = psmm.tile([32, 512], F32, tag="psj")

            def warm_mm(lhs32, rhs_f32, nfree=64):
                nc.tensor.matmul(psj[0:32, 0:nfree], lhsT=lhs32,
                                 rhs=rhs_f32[:, 0:nfree],
                                 start=True, stop=True, tile_position=(0, 0))

            for k in range(internal_steps):
                # each col-tile group j accumulates its own m-quarter in its
                # own PSUM bank (bank stride 512 fp32 = 2 KiB)
                psa = psmm.tile([128, 4, 512], F32, tag="psa")
                for ncnk in range(nch):
                    for j in range(4):
                        nc.tensor.matmul(
                            psa[32 * j:32 * (j + 1), j, 0:mq],
                            lhsT=xcur[:, ncnk * BC:(ncnk + 1) * BC],
                            rhs=atb_r[:, ncnk, j * mq:(j + 1) * mq],
                            start=(ncnk == 0), stop=(ncnk == nch - 1),
                            tile_position=(0, 32 * j))
                # cross-quadrant DVE evictions, fused with the fp8 descale
                # and the +y add (yT is host-transposed to [bc, m]):
                # coupT = psa * (1/A_SCALE) + y^T  ->  (coup + y)^T
                coupT = ew.tile([32, m_loc], F32, tag="coupT")
                for j in range(4):
                    nc.vector.scalar_tensor_tensor(
                        out=coupT[:, j * mq:(j + 1) * mq],
                        in0=psa[32 * j:32 * (j + 1), j, 0:mq],
                        scalar=(1.0 / A_SCALE if USE_FP8 else 1.0),
                        in1=ylt_v[:, j * mq:(j + 1) * mq],
                        op0=ALU.mult, op1=ALU.add)
                # PE transposes -> yt = coup + y in [m partitions, bc] (PSUM)
                psb = psf.tile([128, fw], F32)
                for mc in range(mch):
                    nc.tensor.transpose(psb[:, mc * BC:(mc + 1) * BC],
                                        coupT[:, mc * 128:(mc + 1) * 128],
                                        id32_sb)
                yt = psb
                pr_t = ew.tile([128, fw], F32, tag="pr_t")
                nc.vector.tensor_mul(pr_t, xloc, yt)
                sim = ew.tile([128, fw // 2], F32, tag="sim")
                nc.vector.tensor_reduce(
                    sim, pr_t.rearrange("p (g two) -> p g two", two=2),
                    axis=AXX, op=ALU.add)
                xl3 = xloc.rearrange("p (g two) -> p g two", two=2)
                tmp = ew.tile([128, fw], F32, tag="tmp")
                tm3 = tmp.rearrange("p (g two) -> p g two", two=2)
                proj = ew.tile([128, fw], F32, tag="proj")
                # tmp = sim (pair-broadcast) * xloc
                nc.vector.tensor_mul(tm3, xl3, _pairb(sim))
                nc.vector.tensor_sub(proj, yt, tmp)
                # omega rotation: tmp = pair-swapped(xloc) * [+omg, -omg]
                nc.vector.tensor_mul(tm3, _pairswap(xloc), omg_sb.rearrange(
                    "p (g two) -> p g two", two=2))
                tsum = ew.tile([128, fw], F32, tag="tsum")
                nc.vector.tensor_add(tsum, proj, tmp)
                xn_pre = ew.tile([128, fw], F32, tag="xn_pre")
                nc.vector.scalar_tensor_tensor(out=xn_pre, in0=tsum, scalar=gam_sb,
                                               in1=xloc, op0=ALU.mult, op1=ALU.add)
                # write xn directly into the output accumulator slice; the
                # slice doubles as next step's xloc (no extra copy, no
                # per-step DMA)
                xn = outacc[:, (k % q_steps) * fw:((k % q_steps) + 1) * fw]
                rr = pair_normalize(xn_pre, fw // 2, xn, ew)
                xloc = xn
                if k < internal_steps - 1:
                    xbf = ew.tile([128, fw], MM_DT, tag="xbf")
                    nc.scalar.copy(out=xbf, in_=xn)
                    # stage rows in [p, mh] order: 512B contiguous/partition
                    agi = agd.tile([m_loc, BC], MM_DT, tag="agi")
                    nc.sync.dma_start(
                        out=agi.rearrange("(p mh) c -> p mh c", p=128),
                        in_=xbf.rearrange("p (mh c) -> p mh c", c=BC))
                    ago = agd.tile([n, BC], MM_DT, tag="ago")
                    if "coll" not in ablate:
                        nc.gpsimd.collective_compute(
                            "AllGather", ALU.bypass, replica_groups=rg,
                            ins=[agi.opt()], outs=[ago.opt()])
                    # keep-warm fillers: two hooked on vector intermediates
                    # (fire mid-chain), a chain hooked on xn/xbf that spans the
                    # staging+gather window (WAW on psj serializes them, each
                    # ~0.3-0.4us of PE activity)
                    # trimmed: a long junk chain sits ahead of next step's
                    # matmuls in the PE FIFO (WAW-serialized ~0.4us each) and
                    # can outlive the gather window, delaying real work
                    if "warm" not in ablate:
                        warm_mm(sim[:, 0:32], y_loc)
                        warm_mm(rr[:, 0:32], y_loc)
                        for _ in range(6):
                            warm_mm(xn[:, 0:32], y_loc, nfree=256)
                    # reload in two halves so the second step-half's matmuls
                    # pipeline behind the first half's landing
                    xnew = state.tile([128, nch * BC], MM_DT, tag="xcur")
                    half = n // 2
                    for hh in range(2):
                        nc.sync.dma_start(
                            out=xnew[:, hh * (nch // 2) * BC:
                                     (hh + 1) * (nch // 2) * BC].rearrange(
                                "p (r mh c) -> p r mh c", mh=mch, c=BC),
                            in_=ago[hh * half:(hh + 1) * half, :].rearrange(
                                "(r p mh) c -> p r mh c", p=128, mh=mch))
                    xcur = xnew

            # single contiguous output DMA (4KB/partition)
            nc.sync.dma_start(out=out_o, in_=outacc)

    nc.compile()
    nc.m = get_hw_module(nc.m)
    return nc


def make_inputs(x, c, sc, gn_w, gn_b, conn_w, omg_param, gamma,
                n=N_FULL, ncores=N_CORES):
    """Host-side marshalling: per-core input dicts in SBUF-ready layouts."""
    m_loc = n // ncores
    mch = m_loc // 128
    nch = n // 128
    fw = mch * BC
    bf16 = ml_dtypes.bfloat16

    x = np.asarray(x, np.float32)
    c = np.asarray(c, np.float32)

    # --- groupnorm(c) with C//2 groups over (2 channels, N), torch semantics
    g = c.reshape(B, C // 2, 2, n).astype(np.float64)
    mu = g.mean(axis=(2, 3), keepdims=True)
    var = g.var(axis=(2, 3), keepdims=True)
    gn = ((g - mu) / np.sqrt(var + GN_EPS)).reshape(B, C, n)
    y = (gn * gn_w.astype(np.float64)[None, :, None]
         + gn_b.astype(np.float64)[None, :, None]).astype(np.float32)
    # [B, C, N] -> [N, B*C]
    y_t = np.ascontiguousarray(y.transpose(2, 0, 1).reshape(n, BC))

    # --- x0 = normalize(swapaxes(x, 1, 2)) -> [N, B*C]
    xt = x.transpose(0, 2, 1)  # [B, N, C]
    v = xt.reshape(B, n, C // 2, 2)
    nrm = np.sqrt((v * v).sum(axis=-1, keepdims=True))
    x0 = (v / (nrm + NRM_EPS)).reshape(B, n, C)
    x0t = np.ascontiguousarray(x0.transpose(1, 0, 2).reshape(n, BC))

    # full transposed x0 in chunk layout [p, t, bc]
    mm_np = ml_dtypes.float8_e4m3 if USE_FP8 else bf16
    xg = np.ascontiguousarray(
        x0t.reshape(nch, 128, BC).transpose(1, 0, 2)).astype(mm_np)
    xg = xg.reshape(128, nch * BC)

    # --- omega row: [mh*32 + b*16 + 2g] = omg_g, [.. 2g+1] = -omg_g
    omg = np.abs(omg_param.astype(np.float32)[:, 0])  # [C//2]
    row = np.empty(BC, np.float32)
    for b in range(B):
        for gg in range(C // 2):
            row[b * C + 2 * gg] = omg[gg]
            row[b * C + 2 * gg + 1] = -omg[gg]
    omg_full = np.broadcast_to(np.tile(row, mch)[None, :], (128, fw))

    # --- A = sc[0] * conn_w, premultiplied + cast, per-core A^T shard in
    # partition-contiguous layout [p, t, m] (one 128x128KB DMA per core).
    # fp8: scale so typical ~1e-4 entries land in e4m3's normal range.
    A_f32 = np.asarray(sc[0], np.float32) * np.asarray(conn_w, np.float32)
    if USE_FP8:
        A_bf = (A_f32 * A_SCALE).astype(mm_np)
    else:
        A_bf = A_f32.astype(bf16)

    sw = 3 * fw + 1 + 32 + m_loc
    in_maps = []
    for r in range(ncores):
        sl = slice(r * m_loc, (r + 1) * m_loc)
        at3 = np.ascontiguousarray(
            A_bf[sl].reshape(m_loc, nch, 128).transpose(2, 1, 0))
        xl3 = x0t[sl].reshape(mch, 128, BC).transpose(1, 0, 2)
        yl3 = y_t[sl].reshape(mch, 128, BC).transpose(1, 0, 2)
        small = np.zeros((128, sw), np.float32)
        small[:, 0:fw] = xl3.reshape(128, fw)
        small[:, fw:2 * fw] = yl3.reshape(128, fw)
        small[:, 2 * fw:3 * fw] = omg_full
        small[:, 3 * fw] = float(np.asarray(gamma).reshape(-1)[0])
        small[0:32, 3 * fw + 1:3 * fw + 33] = np.eye(32, dtype=np.float32)
        small[0:32, 3 * fw + 33:3 * fw + 33 + m_loc] = y_t[sl].T
        in_maps.append(dict(
            xg_i=xg,
            at_i=at3.reshape(128, nch * m_loc),
            small_i=small,
        ))
    return in_maps


def unshard_output(outs, n=N_FULL, ncores=N_CORES, q_steps=Q_STEPS):
    """Per-core out_o [128, q*fw] f32 -> full [Q, B, N, C]."""
    m_loc = n // ncores
    mch = m_loc // 128
    parts = []
    for r in range(ncores):
        arr = np.asarray(outs[r]).reshape(128, q_steps, mch, B, C)
        # [p, k, mh, b, c] -> [k, b, mh, p, c] ; slab row m = mh*128 + p
        parts.append(np.ascontiguousarray(
            arr.transpose(1, 3, 2, 0, 4)).reshape(q_steps, B, m_loc, C))
    return np.ascontiguousarray(np.concatenate(parts, axis=2), dtype=np.float32)


_PROGRAM_CACHE = {}


def get_program(n=N_FULL, ncores=N_CORES, q_steps=Q_STEPS):
    key = (n, ncores, q_steps)
    if key not in _PROGRAM_CACHE:
        _PROGRAM_CACHE[key] = build_program(n, ncores, q_steps)
    return _PROGRAM_CACHE[key]


def kernel(x, c, sc, gn_w, gn_b, conn_w, omg_param, gamma, Q):
    assert int(Q) == Q_STEPS
    x = np.asarray(x); c = np.asarray(c); sc = np.asarray(sc)
    gn_w = np.asarray(gn_w); gn_b = np.asarray(gn_b)
    conn_w = np.asarray(conn_w); omg_param = np.asarray(omg_param)
    gamma = np.asarray(gamma)
    n = x.shape[2]
    nc = get_program(n, N_CORES, Q_STEPS)
    in_maps = make_inputs(x, c, sc, gn_w, gn_b, conn_w, omg_param, gamma,
                          n=n, ncores=N_CORES)
    res = run_bass_kernel_spmd(nc, in_maps, core_ids=list(range(N_CORES)))
    outs = [res.results[r]["out_o"] for r in range(N_CORES)]
    return unshard_output(outs, n=n)


# revision 41
# speedup vs baseline: 1.0419x; 1.0419x over previous
"""KMLayer (Kuramoto oscillator layer) on 8 Trainium2 NeuronCores via Bass/Tile.

Strategy (row-sharded, output-node parallel):
  - A = sc[0] * conn_w  [N,N] is row-sharded: core r owns rows m in
    [r*M_LOC, (r+1)*M_LOC).  The shard is premultiplied on the host, scaled
    by A_SCALE and cast to fp8-e4m3 (entries are ~1e-4; the descale rides
    the fused yt add), laid out partition-contiguous as A^T
    [n_lo=128 part, n_hi, m] so the kernel loads it with max-efficiency
    DMAs and keeps it RESIDENT in SBUF (8 MB/core).
  - The host also precomputes the cheap O(B*C*N) prep: groupnorm(c) -> y,
    normalize/transpose of x0, in the exact SBUF layouts the kernel uses.
  - Each Euler step: coup.T = X^T-stationary matmul over the SBUF-resident
    A^T shard (4-way PE column tiling), PE transposes fold [bc, m] ->
    [m, bc], then the per-row update (tangent projection, omega rotation,
    pair renormalize) runs on DVE/ACT for the local rows only, fused with
    zero-stride / pair-swap AP views.
  - The new local state slab (cast to fp8) is AllGather'd across the 8
    cores each step so every core has the full X for the next matmul.
  - Keep-warm matmuls hooked on vector-chain intermediates bridge the
    collective gap so the PE HAM clock-gate stays at full rate.
State is carried in fp32; only the matmul operands (A, gathered X) are fp8.
"""

import numpy as np
import ml_dtypes

import concourse.bass as bass
import concourse.mybir as mybir
import concourse.tile as tile
from concourse import bacc
from concourse.bass_utils import run_bass_kernel_spmd
from concourse.bass_interp import get_hw_module

F32 = mybir.dt.float32
BF16 = mybir.dt.bfloat16
F8 = mybir.dt.float8e4
ALU = mybir.AluOpType
ACTF = mybir.ActivationFunctionType
AXX = mybir.AxisListType.X

# fp8 matmul operands: halves the 16MB/exec A^T reload and the per-step
# gather payload.  A is scaled by A_SCALE on the host so its ~1e-4-magnitude
# entries sit in e4m3's normal range; the descale rides the fused yt add.
USE_FP8 = True
A_SCALE = 1024.0
MM_DT = F8 if USE_FP8 else BF16

N_CORES = 8
B, C, N_FULL = 2, 16, 8192
BC = B * C  # 32
Q_STEPS = 8
GN_EPS = 1e-5
NRM_EPS = 1e-6


def _raw(x):
    """Underlying 2D AP of a tile or AP."""
    return x[:, :]


def _pairb(x):
    """[p, g] -> [p, g, 2] zero-stride broadcast over the pair dim."""
    ap = _raw(x)
    return bass.AP(tensor=ap.tensor, offset=ap.offset, ap=list(ap.ap) + [[0, 2]])


def _pairswap(x):
    """[p, 2g] -> [p, g, 2] view with each pair's elements swapped."""
    ap = _raw(x)
    g = ap.ap[-1][1] // 2
    return bass.AP(tensor=ap.tensor, offset=ap.offset + 1,
                   ap=[list(ap.ap[0]), [2, g], [-1, 2]])


def build_program(n=N_FULL, ncores=N_CORES, q_steps=Q_STEPS, internal_steps=None,
                  ablate=()):
    # internal_steps: timing probe only — run that many Euler iterations
    # (output written for the first q_steps). Default = q_steps.
    # ablate: timing-sim probe only — {"coll", "warm", "stage"} to skip parts.
    if internal_steps is None:
        internal_steps = q_steps
    m_loc = n // ncores            # rows owned per core (1024)
    mch = m_loc // 128             # 128-row chunks per core (8)
    nch = n // 128                 # 128-col contraction chunks (64)
    fw = mch * BC                  # per-step local free width (256)
    mq = m_loc // 4                # m-range per PE column-tile group (256)
    rg = [list(range(ncores))]

    nc = bacc.Bacc("TRN2", target_bir_lowering=False, debug=False,
                   enable_asserts=False, num_devices=ncores)

    # ---- I/O (all pre-marshalled on host into SBUF-ready layouts) ----
    # small_i packs xl | y | omg(replicated) | gam | id32 | yT into one
    # tensor: fewer per-execute buffer bindings.  yT ([bc, m] layout, 32
    # partitions) feeds the eviction-fused "coup*descale + y" add.
    sw = 3 * fw + 1 + 32 + m_loc
    at_i = nc.dram_tensor("at_i", [128, nch * m_loc], MM_DT, kind="ExternalInput").ap()
    xg_i = nc.dram_tensor("xg_i", [128, nch * BC], MM_DT, kind="ExternalInput").ap()
    small_i = nc.dram_tensor("small_i", [128, sw], F32, kind="ExternalInput").ap()
    out_o = nc.dram_tensor("out_o", [128, q_steps * fw], F32,
                           kind="ExternalOutput").ap()

    with tile.TileContext(nc) as tc:
        with tc.tile_pool(name="consts", bufs=1) as consts, \
             tc.tile_pool(name="atbp", bufs=1) as atbp, \
             tc.tile_pool(name="state", bufs=2) as state, \
             tc.tile_pool(name="psmm", bufs=1, space="PSUM") as psmm, \
             tc.tile_pool(name="psf", bufs=2, space="PSUM") as psf, \
             tc.tile_pool(name="ew", bufs=2) as ew, \
             tc.tile_pool(name="agd", bufs=2, space="DRAM") as agd:

            # ---------------- constants & state loads ----------------
            small = consts.tile([128, sw], F32)
            nc.sync.dma_start(out=small, in_=small_i)
            xl_v = small[:, 0:fw]
            y_loc = small[:, fw:2 * fw]
            omg_sb = small[:, 2 * fw:3 * fw]
            gam_sb = small[:, 3 * fw:3 * fw + 1]
            id32_sb = small[0:32, 3 * fw + 1:3 * fw + 33]
            ylt_v = small[0:32, 3 * fw + 33:3 * fw + 33 + m_loc]
            eps6_sb = consts.tile([128, 1], F32)
            nc.vector.memset(eps6_sb, 2.0 * NRM_EPS)

            # persistent A^T shard [n_lo=128 part, (n_hi)(m_loc) free] bf16
            # (split into 8 DMAs so first-step matmuls overlap the load)
            atb = atbp.tile([128, nch * m_loc], MM_DT)
            ldc = nch * m_loc // 8
            for j in range(8):
                nc.sync.dma_start(out=atb[:, j * ldc:(j + 1) * ldc],
                                  in_=at_i[:, j * ldc:(j + 1) * ldc])
            atb_r = atb.rearrange("p (t m) -> p t m", m=m_loc)

            xcur = state.tile([128, nch * BC], MM_DT, tag="xcur")
            nc.sync.dma_start(out=xcur, in_=xg_i)
            xloc = state.tile([128, fw], F32, tag="xloc")
            nc.vector.tensor_copy(xloc, xl_v)
            # all steps' xn accumulate here; one contiguous DMA at the end
            outacc = consts.tile([128, q_steps * fw], F32)

            def pair_normalize(src, npairs, dst, pool):
                """dst = src / (||pair||+eps), src/dst [128, 2*npairs].
                sqrt(ss + 2e-6) ~= sqrt(ss) + 1e-6 near ss=1 (error << tol)."""
                sq = pool.tile([128, 2 * npairs], F32, tag="pn_sq")
                nc.vector.tensor_mul(sq, src, src)
                ss = pool.tile([128, npairs], F32, tag="pn_ss")
                nc.vector.tensor_reduce(
                    ss, sq.rearrange("p (g two) -> p g two", two=2),
                    axis=AXX, op=ALU.add)
                nr = pool.tile([128, npairs], F32, tag="pn_nr")
                nc.scalar.activation(out=nr, in_=ss, func=ACTF.Sqrt,
                                     bias=eps6_sb)
                rr = pool.tile([128, npairs], F32, tag="pn_rr")
                nc.vector.reciprocal_approx_fast(out=rr, in_=nr)
                sv = src.rearrange("p (g two) -> p g two", two=2)
                dv = dst.rearrange("p (g two) -> p g two", two=2)
                nc.vector.tensor_mul(dv, sv, _pairb(rr))
                return rr

            # ---------------- Euler steps ----------------
            if internal_steps == 0:
                for k in range(q_steps):
                    nc.vector.tensor_copy(outacc[:, k * fw:(k + 1) * fw], xloc)

            # scratch PSUM bank for keep-warm matmuls (never read back):
            # the PE HAM clock-gate re-throttles to 1.2 GHz after ~3.4us of
            # idle; the inter-step DVE/collective gap is ~15us, so without
            # fillers every step's matmuls run cold.  Junk matmuls hooked on
            # vector-chain intermediates + the gather staging keep it warm.
            psj = psmm.tile([32, 512], F32, tag="psj")

            def warm_mm(lhs32, rhs_f32, nfree=64):
                nc.tensor.matmul(psj[0:32, 0:nfree], lhsT=lhs32,
                                 rhs=rhs_f32[:, 0:nfree],
                                 start=True, stop=True, tile_position=(0, 0))

            for k in range(internal_steps):
                # each col-tile group j accumulates its own m-quarter in its
                # own PSUM bank (bank stride 512 fp32 = 2 KiB)
                psa = psmm.tile([128, 4, 512], F32, tag="psa")
                for ncnk in range(nch):
                    for j in range(4):
                        nc.tensor.matmul(
                            psa[32 * j:32 * (j + 1), j, 0:mq],
                            lhsT=xcur[:, ncnk * BC:(ncnk + 1) * BC],
                            rhs=atb_r[:, ncnk, j * mq:(j + 1) * mq],
                            start=(ncnk == 0), stop=(ncnk == nch - 1),
                            tile_position=(0, 32 * j))
                # cross-quadrant DVE evictions, fused with the fp8 descale
                # and the +y add (yT is host-transposed to [bc, m]):
                # coupT = psa * (1/A_SCALE) + y^T  ->  (coup + y)^T
                coupT = ew.tile([32, m_loc], F32, tag="coupT")
                for j in range(4):
                    nc.vector.scalar_tensor_tensor(
                        out=coupT[:, j * mq:(j + 1) * mq],
                        in0=psa[32 * j:32 * (j + 1), j, 0:mq],
                        scalar=(1.0 / A_SCALE if USE_FP8 else 1.0),
                        in1=ylt_v[:, j * mq:(j + 1) * mq],
                        op0=ALU.mult, op1=ALU.add)
                # PE transposes -> yt = coup + y in [m partitions, bc] (PSUM)
                psb = psf.tile([128, fw], F32)
                for mc in range(mch):
                    nc.tensor.transpose(psb[:, mc * BC:(mc + 1) * BC],
                                        coupT[:, mc * 128:(mc + 1) * 128],
                                        id32_sb)
                yt = psb
                pr_t = ew.tile([128, fw], F32, tag="pr_t")
                nc.vector.tensor_mul(pr_t, xloc, yt)
                sim = ew.tile([128, fw // 2], F32, tag="sim")
                nc.vector.tensor_reduce(
                    sim, pr_t.rearrange("p (g two) -> p g two", two=2),
                    axis=AXX, op=ALU.add)
                xl3 = xloc.rearrange("p (g two) -> p g two", two=2)
                tmp = ew.tile([128, fw], F32, tag="tmp")
                tm3 = tmp.rearrange("p (g two) -> p g two", two=2)
                proj = ew.tile([128, fw], F32, tag="proj")
                # tmp = sim (pair-broadcast) * xloc
                nc.vector.tensor_mul(tm3, xl3, _pairb(sim))
                nc.vector.tensor_sub(proj, yt, tmp)
                # omega rotation: tmp = pair-swapped(xloc) * [+omg, -omg]
                nc.vector.tensor_mul(tm3, _pairswap(xloc), omg_sb.rearrange(
                    "p (g two) -> p g two", two=2))
                tsum = ew.tile([128, fw], F32, tag="tsum")
                nc.vector.tensor_add(tsum, proj, tmp)
                xn_pre = ew.tile([128, fw], F32, tag="xn_pre")
                nc.vector.scalar_tensor_tensor(out=xn_pre, in0=tsum, scalar=gam_sb,
                                               in1=xloc, op0=ALU.mult, op1=ALU.add)
                # write xn directly into the output accumulator slice; the
                # slice doubles as next step's xloc (no extra copy, no
                # per-step DMA)
                xn = outacc[:, (k % q_steps) * fw:((k % q_steps) + 1) * fw]
                rr = pair_normalize(xn_pre, fw // 2, xn, ew)
                xloc = xn
                if k < internal_steps - 1:
                    xbf = ew.tile([128, fw], MM_DT, tag="xbf")
                    nc.scalar.copy(out=xbf, in_=xn)
                    # stage rows in [p, mh] order: 512B contiguous/partition
                    agi = agd.tile([m_loc, BC], MM_DT, tag="agi")
                    nc.sync.dma_start(
                        out=agi.rearrange("(p mh) c -> p mh c", p=128),
                        in_=xbf.rearrange("p (mh c) -> p mh c", c=BC))
                    ago = agd.tile([n, BC], MM_DT, tag="ago")
                    if "coll" not in ablate:
                        nc.gpsimd.collective_compute(
                            "AllGather", ALU.bypass, replica_groups=rg,
                            ins=[agi.opt()], outs=[ago.opt()])
                    # keep-warm fillers: two hooked on vector intermediates
                    # (fire mid-chain), a chain hooked on xn/xbf that spans the
                    # staging+gather window (WAW on psj serializes them, each
                    # ~0.3-0.4us of PE activity)
                    if "warm" not in ablate:
                        warm_mm(sim[:, 0:32], y_loc)
                        warm_mm(rr[:, 0:32], y_loc)
                        for _ in range(20):
                            warm_mm(xn[:, 0:32], y_loc, nfree=256)
                    # reload in two halves so the second step-half's matmuls
                    # pipeline behind the first half's landing
                    xnew = state.tile([128, nch * BC], MM_DT, tag="xcur")
                    half = n // 2
                    for hh in range(2):
                        nc.sync.dma_start(
                            out=xnew[:, hh * (nch // 2) * BC:
                                     (hh + 1) * (nch // 2) * BC].rearrange(
                                "p (r mh c) -> p r mh c", mh=mch, c=BC),
                            in_=ago[hh * half:(hh + 1) * half, :].rearrange(
                                "(r p mh) c -> p r mh c", p=128, mh=mch))
                    xcur = xnew

            # output DMA split so the first 7 steps' slab streams out while
            # the last step computes (deps are per-column-range)
            nc.sync.dma_start(out=out_o[:, 0:(q_steps - 1) * fw],
                              in_=outacc[:, 0:(q_steps - 1) * fw])
            nc.sync.dma_start(out=out_o[:, (q_steps - 1) * fw:],
                              in_=outacc[:, (q_steps - 1) * fw:])

    nc.compile()
    nc.m = get_hw_module(nc.m)
    return nc


def make_inputs(x, c, sc, gn_w, gn_b, conn_w, omg_param, gamma,
                n=N_FULL, ncores=N_CORES):
    """Host-side marshalling: per-core input dicts in SBUF-ready layouts."""
    m_loc = n // ncores
    mch = m_loc // 128
    nch = n // 128
    fw = mch * BC
    bf16 = ml_dtypes.bfloat16

    x = np.asarray(x, np.float32)
    c = np.asarray(c, np.float32)

    # --- groupnorm(c) with C//2 groups over (2 channels, N), torch semantics
    g = c.reshape(B, C // 2, 2, n).astype(np.float64)
    mu = g.mean(axis=(2, 3), keepdims=True)
    var = g.var(axis=(2, 3), keepdims=True)
    gn = ((g - mu) / np.sqrt(var + GN_EPS)).reshape(B, C, n)
    y = (gn * gn_w.astype(np.float64)[None, :, None]
         + gn_b.astype(np.float64)[None, :, None]).astype(np.float32)
    # [B, C, N] -> [N, B*C]
    y_t = np.ascontiguousarray(y.transpose(2, 0, 1).reshape(n, BC))

    # --- x0 = normalize(swapaxes(x, 1, 2)) -> [N, B*C]
    xt = x.transpose(0, 2, 1)  # [B, N, C]
    v = xt.reshape(B, n, C // 2, 2)
    nrm = np.sqrt((v * v).sum(axis=-1, keepdims=True))
    x0 = (v / (nrm + NRM_EPS)).reshape(B, n, C)
    x0t = np.ascontiguousarray(x0.transpose(1, 0, 2).reshape(n, BC))

    # full transposed x0 in chunk layout [p, t, bc]
    mm_np = ml_dtypes.float8_e4m3 if USE_FP8 else bf16
    xg = np.ascontiguousarray(
        x0t.reshape(nch, 128, BC).transpose(1, 0, 2)).astype(mm_np)
    xg = xg.reshape(128, nch * BC)

    # --- omega row: [mh*32 + b*16 + 2g] = omg_g, [.. 2g+1] = -omg_g
    omg = np.abs(omg_param.astype(np.float32)[:, 0])  # [C//2]
    row = np.empty(BC, np.float32)
    for b in range(B):
        for gg in range(C // 2):
            row[b * C + 2 * gg] = omg[gg]
            row[b * C + 2 * gg + 1] = -omg[gg]
    omg_full = np.broadcast_to(np.tile(row, mch)[None, :], (128, fw))

    # --- A = sc[0] * conn_w, premultiplied + cast, per-core A^T shard in
    # partition-contiguous layout [p, t, m] (one 128x128KB DMA per core).
    # fp8: scale so typical ~1e-4 entries land in e4m3's normal range.
    A_f32 = np.asarray(sc[0], np.float32) * np.asarray(conn_w, np.float32)
    if USE_FP8:
        A_bf = (A_f32 * A_SCALE).astype(mm_np)
    else:
        A_bf = A_f32.astype(bf16)

    sw = 3 * fw + 1 + 32 + m_loc
    in_maps = []
    for r in range(ncores):
        sl = slice(r * m_loc, (r + 1) * m_loc)
        at3 = np.ascontiguousarray(
            A_bf[sl].reshape(m_loc, nch, 128).transpose(2, 1, 0))
        xl3 = x0t[sl].reshape(mch, 128, BC).transpose(1, 0, 2)
        yl3 = y_t[sl].reshape(mch, 128, BC).transpose(1, 0, 2)
        small = np.zeros((128, sw), np.float32)
        small[:, 0:fw] = xl3.reshape(128, fw)
        small[:, fw:2 * fw] = yl3.reshape(128, fw)
        small[:, 2 * fw:3 * fw] = omg_full
        small[:, 3 * fw] = float(np.asarray(gamma).reshape(-1)[0])
        small[0:32, 3 * fw + 1:3 * fw + 33] = np.eye(32, dtype=np.float32)
        small[0:32, 3 * fw + 33:3 * fw + 33 + m_loc] = y_t[sl].T
        in_maps.append(dict(
            xg_i=xg,
            at_i=at3.reshape(128, nch * m_loc),
            small_i=small,
        ))
    return in_maps


def unshard_output(outs, n=N_FULL, ncores=N_CORES, q_steps=Q_STEPS):
    """Per-core out_o [128, q*fw] f32 -> full [Q, B, N, C]."""
    m_loc = n // ncores
    mch = m_loc // 128
    parts = []
    for r in range(ncores):
        arr = np.asarray(outs[r]).reshape(128, q_steps, mch, B, C)
        # [p, k, mh, b, c] -> [k, b, mh, p, c] ; slab row m = mh*128 + p
        parts.append(np.ascontiguousarray(
            arr.transpose(1, 3, 2, 0, 4)).reshape(q_steps, B, m_loc, C))
    return np.ascontiguousarray(np.concatenate(parts, axis=2), dtype=np.float32)


_PROGRAM_CACHE = {}


def get_program(n=N_FULL, ncores=N_CORES, q_steps=Q_STEPS):
    key = (n, ncores, q_steps)
    if key not in _PROGRAM_CACHE:
        _PROGRAM_CACHE[key] = build_program(n, ncores, q_steps)
    return _PROGRAM_CACHE[key]


def kernel(x, c, sc, gn_w, gn_b, conn_w, omg_param, gamma, Q):
    assert int(Q) == Q_STEPS
    x = np.asarray(x); c = np.asarray(c); sc = np.asarray(sc)
    gn_w = np.asarray(gn_w); gn_b = np.asarray(gn_b)
    conn_w = np.asarray(conn_w); omg_param = np.asarray(omg_param)
    gamma = np.asarray(gamma)
    n = x.shape[2]
    nc = get_program(n, N_CORES, Q_STEPS)
    in_maps = make_inputs(x, c, sc, gn_w, gn_b, conn_w, omg_param, gamma,
                          n=n, ncores=N_CORES)
    res = run_bass_kernel_spmd(nc, in_maps, core_ids=list(range(N_CORES)))
    outs = [res.results[r]["out_o"] for r in range(N_CORES)]
    return unshard_output(outs, n=n)
